# revision 1
# baseline (speedup 1.0000x reference)
"""ExpSyn kernel: diagonal linear recurrence isyn_t = beta*isyn_{t-1} + x_t.

Strategy:
  - Host: transpose data (B,T,N) -> (B,N,T) so time is contiguous per channel.
  - Shard batch over 8 cores (2 batches/core -> 1024 rows of length T=4096).
  - Device: per 128-row tile, DMA load [128, 4096], run the DVE
    tensor_tensor_scan (state = beta*state + x along the free dim), DMA store.
  - Host: gather, transpose back to (B,T,N).
"""

import numpy as np

DT = 1e-4
B, T, N = 16, 4096, 512
NCORES = 8
BLOC = B // NCORES          # 2 batches per core
ROWS = BLOC * N             # 1024 scan rows per core
NG = N // 128               # 4 channel groups of 128
NTILES = ROWS // 128        # 8 tiles per core

_cached = None


def _build():
    """Build + compile the single-core Bass program (run SPMD on 8 cores)."""
    import concourse.bacc as bacc
    import concourse.mybir as mybir
    from concourse import tile

    nc = bacc.Bacc("TRN2", debug=False, num_devices=NCORES)
    f32 = mybir.dt.float32

    x = nc.dram_tensor("x", [ROWS, T], f32, kind="ExternalInput")
    beta_d = nc.dram_tensor("beta", [NG, 128, 1], f32, kind="ExternalInput")
    y = nc.dram_tensor("y", [ROWS, T], f32, kind="ExternalOutput")

    with tile.TileContext(nc) as tc:
        with (
            tc.tile_pool(name="const", bufs=1) as cpool,
            tc.tile_pool(name="xin", bufs=3) as xpool,
            tc.tile_pool(name="yout", bufs=3) as ypool,
        ):
            betas = []
            for g in range(NG):
                bsb = cpool.tile([128, 1], f32, tag=f"beta{g}", name=f"beta_sb{g}")
                nc.sync.dma_start(out=bsb[:, :], in_=beta_d[g])
                betas.append(bsb)

            for k in range(NTILES):
                g = k % NG
                xt = xpool.tile([128, T], f32, tag="xt", name=f"xt{k}")
                nc.sync.dma_start(out=xt[:, :], in_=x[k * 128:(k + 1) * 128, :])
                yt = ypool.tile([128, T], f32, tag="yt", name=f"yt{k}")
                nc.vector.tensor_tensor_scan(
                    yt[:, :],
                    betas[g][:, 0:1].broadcast_to([128, T]),
                    xt[:, :],
                    0.0,
                    mybir.AluOpType.mult,
                    mybir.AluOpType.add,
                )
                nc.sync.dma_start(out=y[k * 128:(k + 1) * 128, :], in_=yt[:, :])

    nc.compile()
    return nc


def _get_nc():
    global _cached
    if _cached is None:
        _cached = _build()
    return _cached


def _make_in_maps(data, tau_syn):
    beta = np.exp(-DT / tau_syn.astype(np.float64)).astype(np.float32)  # (1, N)
    beta_g = np.ascontiguousarray(beta.reshape(NG, 128, 1))
    # (B, T, N) -> (B, N, T), batch-sharded across cores
    xt = np.ascontiguousarray(np.asarray(data, dtype=np.float32).transpose(0, 2, 1))
    xt = xt.reshape(NCORES, ROWS, T)
    return [{"x": xt[c], "beta": beta_g} for c in range(NCORES)]


def kernel(data, tau_syn):
    from concourse.bass_utils import run_bass_kernel_spmd

    nc = _get_nc()
    in_maps = _make_in_maps(data, tau_syn)
    res = run_bass_kernel_spmd(nc, in_maps, list(range(NCORES)))
    out = np.stack([res.results[c]["y"] for c in range(NCORES)])  # (8, ROWS, T)
    out = out.reshape(B, N, T).transpose(0, 2, 1)  # (B, T, N)
    return np.ascontiguousarray(out)


# revision 11
# speedup vs baseline: 199.1925x; 199.1925x over previous
"""ExpSyn kernel: diagonal linear recurrence isyn_t = beta*isyn_{t-1} + x_t.

Strategy:
  - Host: transpose data (B,T,N) -> (B,N,T) so time is contiguous per channel.
  - Shard batch over 8 cores (2 batches/core -> 1024 rows of length T=4096).
  - Device: per 128-row block, DMA load, run the DVE tensor_tensor_scan
    (state = beta*state + x along the free/time dim), DMA store.
    First block is loaded/scanned in 512KB chunks (chained via initial=)
    so the DVE starts early; last block stores in chunks so the tail is
    short; middle blocks ride 4MB double-tile DMAs for bandwidth.
  - Host: gather, transpose back to (B,T,N).
"""

import numpy as np

DT = 1e-4
B, T, N = 16, 4096, 512
NCORES = 8
BLOC = B // NCORES          # 2 batches per core
ROWS = BLOC * N             # 1024 scan rows per core
NG = N // 128               # 4 channel groups of 128
NTILES = ROWS // 128        # 8 row-blocks per core

_cached = None


def _build():
    """Build + compile the single-core Bass program (run SPMD on 8 cores)."""
    import concourse.bacc as bacc
    import concourse.mybir as mybir
    from concourse import tile

    nc = bacc.Bacc("TRN2", debug=False, num_devices=NCORES)
    f32 = mybir.dt.float32
    mult, add = mybir.AluOpType.mult, mybir.AluOpType.add

    x = nc.dram_tensor("x", [ROWS, T], f32, kind="ExternalInput")
    beta_d = nc.dram_tensor("beta", [128, NG], f32, kind="ExternalInput")
    y = nc.dram_tensor("y", [ROWS, T], f32, kind="ExternalOutput")

    with tile.TileContext(nc) as tc:
        with (
            tc.tile_pool(name="const", bufs=1) as cpool,
            tc.tile_pool(name="xin", bufs=3) as xpool,
            tc.tile_pool(name="yout", bufs=3) as ypool,
        ):
            # tiny beta DMA rides the ACT ring (idle until the first store,
            # so it lands well before the first scan needs it)
            bsb = cpool.tile([128, NG], f32, name="bsb")
            nc.scalar.dma_start(out=bsb[:, :], in_=beta_d[:, :])

            def bcast(g, n):
                return bsb[:, g:g + 1].broadcast_to([128, n])

            # ---- block 0: chunked loads so the DVE starts ASAP ----
            # geometric chunk sizes: tiny first chunk -> earliest scan start
            bounds = [0, 512, 1024, 2048, T]
            xt0 = xpool.tile([128, T], f32, tag="xt", name="xt0")
            for c in range(len(bounds) - 1):
                lo, hi = bounds[c], bounds[c + 1]
                nc.sync.dma_start(out=xt0[:, lo:hi], in_=x[0:128, lo:hi])
            yt0 = ypool.tile([128, T], f32, tag="yt", name="yt0")
            for c in range(len(bounds) - 1):
                lo, hi = bounds[c], bounds[c + 1]
                init = 0.0 if c == 0 else yt0[:, lo - 1:lo]
                nc.vector.tensor_tensor_scan(
                    yt0[:, lo:hi], bcast(0, hi - lo), xt0[:, lo:hi],
                    init, mult, add)
            nc.scalar.dma_start(out=y[0:128, :], in_=yt0[:, :])

            # ---- blocks 1..6: 2MB load; scan + store in halves so the
            # store stream starts mid-scan and bandwidth stays smooth ----
            H = T // 2
            for k in range(1, NTILES - 1):
                g = k % NG
                xt = xpool.tile([128, T], f32, tag="xt", name=f"xt{k}")
                nc.sync.dma_start(out=xt[:, :], in_=x[k * 128:(k + 1) * 128, :])
                yt = ypool.tile([128, T], f32, tag="yt", name=f"yt{k}")
                nc.vector.tensor_tensor_scan(
                    yt[:, 0:H], bcast(g, H), xt[:, 0:H], 0.0, mult, add)
                nc.scalar.dma_start(out=y[k * 128:(k + 1) * 128, 0:H],
                                    in_=yt[:, 0:H])
                nc.vector.tensor_tensor_scan(
                    yt[:, H:T], bcast(g, H), xt[:, H:T],
                    yt[:, H - 1:H], mult, add)
                nc.scalar.dma_start(out=y[k * 128:(k + 1) * 128, H:T],
                                    in_=yt[:, H:T])

            # ---- block 7: chunked stores so the tail is short ----
            k = NTILES - 1
            xt7 = xpool.tile([128, T], f32, tag="xt", name="xt7")
            nc.sync.dma_start(out=xt7[:, :], in_=x[k * 128:(k + 1) * 128, :])
            yt7 = ypool.tile([128, T], f32, tag="yt", name="yt7")
            g = k % NG
            # uniform chunks so the last store is small (short tail)
            bounds7 = [0, 1024, 2048, 3072, T]
            for c in range(len(bounds7) - 1):
                lo, hi = bounds7[c], bounds7[c + 1]
                init = 0.0 if c == 0 else yt7[:, lo - 1:lo]
                nc.vector.tensor_tensor_scan(
                    yt7[:, lo:hi], bcast(g, hi - lo), xt7[:, lo:hi],
                    init, mult, add)
                nc.scalar.dma_start(out=y[k * 128:(k + 1) * 128, lo:hi],
                                    in_=yt7[:, lo:hi])

    nc.compile()
    return nc


def _get_nc():
    global _cached
    if _cached is None:
        _cached = _build()
    return _cached


def _make_in_maps(data, tau_syn):
    beta = np.exp(-DT / tau_syn.astype(np.float64)).astype(np.float32)  # (1, N)
    beta_g = np.ascontiguousarray(beta.reshape(NG, 128).T)  # (128, NG)
    # (B, T, N) -> (B, N, T), batch-sharded across cores
    xt = np.ascontiguousarray(np.asarray(data, dtype=np.float32).transpose(0, 2, 1))
    xt = xt.reshape(NCORES, ROWS, T)
    return [{"x": xt[c], "beta": beta_g} for c in range(NCORES)]


def kernel(data, tau_syn):
    from concourse.bass_utils import run_bass_kernel_spmd

    nc = _get_nc()
    in_maps = _make_in_maps(data, tau_syn)
    res = run_bass_kernel_spmd(nc, in_maps, list(range(NCORES)))
    out = np.stack([res.results[c]["y"] for c in range(NCORES)])  # (8, ROWS, T)
    out = out.reshape(B, N, T).transpose(0, 2, 1)  # (B, T, N)
    return np.ascontiguousarray(out)


# revision 12
# speedup vs baseline: 226.0944x; 1.1351x over previous
"""ExpSyn kernel: diagonal linear recurrence isyn_t = beta*isyn_{t-1} + x_t.

Strategy:
  - Host: transpose data (B,T,N) -> (B,N,T) so time is contiguous per channel.
  - Shard batch over 8 cores (2 batches/core -> 1024 rows of length T=4096).
  - Device: per 128-row block, DMA load, run the DVE tensor_tensor_scan
    (state = beta*state + x along the free/time dim), DMA store.
    First block is loaded/scanned in 512KB chunks (chained via initial=)
    so the DVE starts early; last block stores in chunks so the tail is
    short; middle blocks ride 4MB double-tile DMAs for bandwidth.
  - Host: gather, transpose back to (B,T,N).
"""

import numpy as np

DT = 1e-4
B, T, N = 16, 4096, 512
NCORES = 8
BLOC = B // NCORES          # 2 batches per core
ROWS = BLOC * N             # 1024 scan rows per core
NG = N // 128               # 4 channel groups of 128
NTILES = ROWS // 128        # 8 row-blocks per core

_cached = None


def _build():
    """Build + compile the single-core Bass program (run SPMD on 8 cores)."""
    import concourse.bacc as bacc
    import concourse.mybir as mybir
    from concourse import tile

    nc = bacc.Bacc("TRN2", debug=False, num_devices=NCORES)
    f32 = mybir.dt.float32
    mult, add = mybir.AluOpType.mult, mybir.AluOpType.add

    x = nc.dram_tensor("x", [ROWS, T], f32, kind="ExternalInput")
    beta_d = nc.dram_tensor("beta", [128, NG], f32, kind="ExternalInput")
    y = nc.dram_tensor("y", [ROWS, T], f32, kind="ExternalOutput")

    with tile.TileContext(nc) as tc:
        with (
            tc.tile_pool(name="const", bufs=1) as cpool,
            tc.tile_pool(name="xin", bufs=3) as xpool,
            tc.tile_pool(name="yout", bufs=3) as ypool,
        ):
            # tiny beta DMA rides the ACT ring (idle until the first store,
            # so it lands well before the first scan needs it)
            bsb = cpool.tile([128, NG], f32, name="bsb")
            nc.scalar.dma_start(out=bsb[:, :], in_=beta_d[:, :])

            def bcast(g, n):
                return bsb[:, g:g + 1].broadcast_to([128, n])

            # ---- block 0: chunked loads so the DVE starts ASAP ----
            # geometric chunk sizes: tiny first chunk -> earliest scan start
            bounds = [0, 512, 1024, 2048, T]
            xt0 = xpool.tile([128, T], f32, tag="xt", name="xt0")
            for c in range(len(bounds) - 1):
                lo, hi = bounds[c], bounds[c + 1]
                nc.sync.dma_start(out=xt0[:, lo:hi], in_=x[0:128, lo:hi])
            yt0 = ypool.tile([128, T], f32, tag="yt", name="yt0")
            for c in range(len(bounds) - 1):
                lo, hi = bounds[c], bounds[c + 1]
                init = 0.0 if c == 0 else yt0[:, lo - 1:lo]
                nc.vector.tensor_tensor_scan(
                    yt0[:, lo:hi], bcast(0, hi - lo), xt0[:, lo:hi],
                    init, mult, add)
            nc.scalar.dma_start(out=y[0:128, :], in_=yt0[:, :])

            # ---- blocks 1..6: 2MB load; scan + store in halves so the
            # store stream starts mid-scan and bandwidth stays smooth ----
            H = T // 2
            for k in range(1, NTILES - 1):
                g = k % NG
                xt = xpool.tile([128, T], f32, tag="xt", name=f"xt{k}")
                nc.sync.dma_start(out=xt[:, :], in_=x[k * 128:(k + 1) * 128, :])
                yt = ypool.tile([128, T], f32, tag="yt", name=f"yt{k}")
                nc.vector.tensor_tensor_scan(
                    yt[:, 0:H], bcast(g, H), xt[:, 0:H], 0.0, mult, add)
                nc.scalar.dma_start(out=y[k * 128:(k + 1) * 128, 0:H],
                                    in_=yt[:, 0:H])
                nc.vector.tensor_tensor_scan(
                    yt[:, H:T], bcast(g, H), xt[:, H:T],
                    yt[:, H - 1:H], mult, add)
                nc.scalar.dma_start(out=y[k * 128:(k + 1) * 128, H:T],
                                    in_=yt[:, H:T])

            # ---- block 7: chunked stores so the tail is short ----
            k = NTILES - 1
            xt7 = xpool.tile([128, T], f32, tag="xt", name="xt7")
            nc.sync.dma_start(out=xt7[:, :], in_=x[k * 128:(k + 1) * 128, :])
            yt7 = ypool.tile([128, T], f32, tag="yt", name="yt7")
            g = k % NG
            # uniform chunks so the last store is small (short tail)
            bounds7 = [0, 1024, 2048, 3072, T]
            for c in range(len(bounds7) - 1):
                lo, hi = bounds7[c], bounds7[c + 1]
                init = 0.0 if c == 0 else yt7[:, lo - 1:lo]
                nc.vector.tensor_tensor_scan(
                    yt7[:, lo:hi], bcast(g, hi - lo), xt7[:, lo:hi],
                    init, mult, add)
                nc.scalar.dma_start(out=y[k * 128:(k + 1) * 128, lo:hi],
                                    in_=yt7[:, lo:hi])

    nc.compile()
    return nc


def _get_nc():
    global _cached
    if _cached is None:
        _cached = _build()
    return _cached


def _make_in_maps(data, tau_syn):
    tau = np.asarray(tau_syn, dtype=np.float64)
    beta = np.exp(-DT / tau).astype(np.float32)  # (1, N)
    beta_g = np.ascontiguousarray(beta.reshape(NG, 128).T)  # (128, NG)
    # (B, T, N) -> (B, N, T), batch-sharded across cores
    xt = np.ascontiguousarray(np.asarray(data, dtype=np.float32).transpose(0, 2, 1))
    xt = xt.reshape(NCORES, ROWS, T)
    return [{"x": xt[c], "beta": beta_g} for c in range(NCORES)]


def kernel(data, tau_syn):
    from concourse.bass_utils import run_bass_kernel_spmd

    nc = _get_nc()
    in_maps = _make_in_maps(data, tau_syn)
    res = run_bass_kernel_spmd(nc, in_maps, list(range(NCORES)))
    out = np.stack([res.results[c]["y"] for c in range(NCORES)])  # (8, ROWS, T)
    out = out.reshape(B, N, T).transpose(0, 2, 1)  # (B, T, N)
    return np.ascontiguousarray(out)


# revision 16
# speedup vs baseline: 227.8951x; 1.0080x over previous
"""ExpSyn kernel: diagonal linear recurrence isyn_t = beta*isyn_{t-1} + x_t.

Strategy:
  - Host: transpose data (B,T,N) -> (B,N,T) so time is contiguous per channel.
  - Shard batch over 8 cores (2 batches/core -> 1024 rows of length T=4096).
  - Device: per 128-row block, 2MB DMA load (nc.sync ring), DVE
    tensor_tensor_scan (state = beta*state + x along the free/time dim),
    2MB DMA store (nc.scalar ring — separate HWDGE FIFO so stores never
    head-of-line block loads). First block loads in geometric chunks
    (chained via initial=) so the DVE starts early; middle blocks scan/
    store in halves to smooth store bandwidth; last block stores in
    chunks so the tail is short.
  - Host: gather, transpose back to (B,T,N).
"""

import numpy as np

DT = 1e-4
B, T, N = 16, 4096, 512
NCORES = 8
BLOC = B // NCORES          # 2 batches per core
ROWS = BLOC * N             # 1024 scan rows per core
NG = N // 128               # 4 channel groups of 128
NTILES = ROWS // 128        # 8 row-blocks per core

_cached = None


def _build():
    """Build + compile the single-core Bass program (run SPMD on 8 cores)."""
    import concourse.bacc as bacc
    import concourse.mybir as mybir
    from concourse import tile

    nc = bacc.Bacc("TRN2", debug=False, num_devices=NCORES)
    f32 = mybir.dt.float32
    mult, add = mybir.AluOpType.mult, mybir.AluOpType.add

    x = nc.dram_tensor("x", [ROWS, T], f32, kind="ExternalInput")
    beta_d = nc.dram_tensor("beta", [128, NG], f32, kind="ExternalInput")
    y = nc.dram_tensor("y", [ROWS, T], f32, kind="ExternalOutput")

    with tile.TileContext(nc) as tc:
        with (
            tc.tile_pool(name="const", bufs=1) as cpool,
            tc.tile_pool(name="xin", bufs=4) as xpool,
            tc.tile_pool(name="yout", bufs=4) as ypool,
        ):
            # tiny beta DMA rides the ACT ring (idle until the first store,
            # so it lands well before the first scan needs it)
            bsb = cpool.tile([128, NG], f32, name="bsb")
            nc.scalar.dma_start(out=bsb[:, :], in_=beta_d[:, :])

            def bcast(g, n):
                return bsb[:, g:g + 1].broadcast_to([128, n])

            # ---- block 0: chunked loads so the DVE starts ASAP ----
            # geometric chunk sizes: tiny first chunk -> earliest scan start
            bounds = [0, 128, 512, 1536, T]
            xt0 = xpool.tile([128, T], f32, tag="xt", name="xt0")
            for c in range(len(bounds) - 1):
                lo, hi = bounds[c], bounds[c + 1]
                nc.sync.dma_start(out=xt0[:, lo:hi], in_=x[0:128, lo:hi])
            yt0 = ypool.tile([128, T], f32, tag="yt", name="yt0")
            for c in range(len(bounds) - 1):
                lo, hi = bounds[c], bounds[c + 1]
                init = 0.0 if c == 0 else yt0[:, lo - 1:lo]
                nc.vector.tensor_tensor_scan(
                    yt0[:, lo:hi], bcast(0, hi - lo), xt0[:, lo:hi],
                    init, mult, add)
            nc.scalar.dma_start(out=y[0:128, :], in_=yt0[:, :])

            # ---- blocks 1..6: 2MB load; scan + store in halves so the
            # store stream starts mid-scan and bandwidth stays smooth ----
            H = T // 2
            for k in range(1, NTILES - 1):
                g = k % NG
                xt = xpool.tile([128, T], f32, tag="xt", name=f"xt{k}")
                nc.sync.dma_start(out=xt[:, :], in_=x[k * 128:(k + 1) * 128, :])
                yt = ypool.tile([128, T], f32, tag="yt", name=f"yt{k}")
                nc.vector.tensor_tensor_scan(
                    yt[:, 0:H], bcast(g, H), xt[:, 0:H], 0.0, mult, add)
                nc.scalar.dma_start(out=y[k * 128:(k + 1) * 128, 0:H],
                                    in_=yt[:, 0:H])
                nc.vector.tensor_tensor_scan(
                    yt[:, H:T], bcast(g, H), xt[:, H:T],
                    yt[:, H - 1:H], mult, add)
                nc.scalar.dma_start(out=y[k * 128:(k + 1) * 128, H:T],
                                    in_=yt[:, H:T])

            # ---- block 7: chunked stores so the tail is short ----
            k = NTILES - 1
            xt7 = xpool.tile([128, T], f32, tag="xt", name="xt7")
            nc.sync.dma_start(out=xt7[:, :], in_=x[k * 128:(k + 1) * 128, :])
            yt7 = ypool.tile([128, T], f32, tag="yt", name="yt7")
            g = k % NG
            # shrinking chunks so the very last store is only 256KB
            bounds7 = [0, 1536, 2560, 3584, T]
            for c in range(len(bounds7) - 1):
                lo, hi = bounds7[c], bounds7[c + 1]
                init = 0.0 if c == 0 else yt7[:, lo - 1:lo]
                nc.vector.tensor_tensor_scan(
                    yt7[:, lo:hi], bcast(g, hi - lo), xt7[:, lo:hi],
                    init, mult, add)
                nc.scalar.dma_start(out=y[k * 128:(k + 1) * 128, lo:hi],
                                    in_=yt7[:, lo:hi])

    nc.compile()
    return nc


def _get_nc():
    global _cached
    if _cached is None:
        _cached = _build()
    return _cached


def _make_in_maps(data, tau_syn):
    tau = np.asarray(tau_syn, dtype=np.float64)
    beta = np.exp(-DT / tau).astype(np.float32)  # (1, N)
    beta_g = np.ascontiguousarray(beta.reshape(NG, 128).T)  # (128, NG)
    # (B, T, N) -> (B, N, T), batch-sharded across cores
    xt = np.ascontiguousarray(np.asarray(data, dtype=np.float32).transpose(0, 2, 1))
    xt = xt.reshape(NCORES, ROWS, T)
    return [{"x": xt[c], "beta": beta_g} for c in range(NCORES)]


def kernel(data, tau_syn):
    from concourse.bass_utils import run_bass_kernel_spmd

    nc = _get_nc()
    in_maps = _make_in_maps(data, tau_syn)
    res = run_bass_kernel_spmd(nc, in_maps, list(range(NCORES)))
    out = np.stack([res.results[c]["y"] for c in range(NCORES)])  # (8, ROWS, T)
    out = out.reshape(B, N, T).transpose(0, 2, 1)  # (B, T, N)
    return np.ascontiguousarray(out)


# revision 18
# speedup vs baseline: 229.8595x; 1.0086x over previous
"""ExpSyn kernel: diagonal linear recurrence isyn_t = beta*isyn_{t-1} + x_t.

Strategy:
  - Host: transpose data (B,T,N) -> (B,N,T) so time is contiguous per channel.
  - Shard batch over 8 cores (2 batches/core -> 1024 rows of length T=4096).
  - Device: per 128-row block, 2MB DMA load (nc.sync ring), DVE
    tensor_tensor_scan (state = beta*state + x along the free/time dim),
    2MB DMA store (nc.scalar ring — separate HWDGE FIFO so stores never
    head-of-line block loads). First block loads in geometric chunks
    (chained via initial=) so the DVE starts early; middle blocks scan/
    store in halves to smooth store bandwidth; last block stores in
    chunks so the tail is short.
  - Host: gather, transpose back to (B,T,N).
"""

import numpy as np

DT = 1e-4
B, T, N = 16, 4096, 512
NCORES = 8
BLOC = B // NCORES          # 2 batches per core
ROWS = BLOC * N             # 1024 scan rows per core
NG = N // 128               # 4 channel groups of 128
NTILES = ROWS // 128        # 8 row-blocks per core

_cached = None


def _build():
    """Build + compile the single-core Bass program (run SPMD on 8 cores)."""
    import concourse.bacc as bacc
    import concourse.mybir as mybir
    from concourse import tile

    nc = bacc.Bacc("TRN2", debug=False, num_devices=NCORES)
    f32 = mybir.dt.float32
    mult, add = mybir.AluOpType.mult, mybir.AluOpType.add

    x = nc.dram_tensor("x", [ROWS, T], f32, kind="ExternalInput")
    beta_d = nc.dram_tensor("beta", [128, NG], f32, kind="ExternalInput")
    y = nc.dram_tensor("y", [ROWS, T], f32, kind="ExternalOutput")

    with tile.TileContext(nc) as tc:
        with (
            tc.tile_pool(name="const", bufs=1) as cpool,
            tc.tile_pool(name="xin", bufs=4) as xpool,
            tc.tile_pool(name="yout", bufs=4) as ypool,
        ):
            # tiny beta DMA rides the ACT ring (idle until the first store,
            # so it lands well before the first scan needs it)
            bsb = cpool.tile([128, NG], f32, name="bsb")
            nc.scalar.dma_start(out=bsb[:, :], in_=beta_d[:, :])

            def bcast(g, n):
                return bsb[:, g:g + 1].broadcast_to([128, n])

            # ---- block 0: chunked loads so the DVE starts ASAP ----
            # geometric chunk sizes: tiny first chunk -> earliest scan start
            bounds = [0, 128, 512, 1536, T]
            xt0 = xpool.tile([128, T], f32, tag="xt", name="xt0")
            for c in range(len(bounds) - 1):
                lo, hi = bounds[c], bounds[c + 1]
                nc.sync.dma_start(out=xt0[:, lo:hi], in_=x[0:128, lo:hi])
            yt0 = ypool.tile([128, T], f32, tag="yt", name="yt0")
            for c in range(len(bounds) - 1):
                lo, hi = bounds[c], bounds[c + 1]
                init = 0.0 if c == 0 else yt0[:, lo - 1:lo]
                nc.vector.tensor_tensor_scan(
                    yt0[:, lo:hi], bcast(0, hi - lo), xt0[:, lo:hi],
                    init, mult, add)
            nc.scalar.dma_start(out=y[0:128, :], in_=yt0[:, :])

            # ---- blocks 1..6: 2MB load; scan + store in halves so the
            # store stream starts mid-scan and bandwidth stays smooth ----
            H = T // 2
            for k in range(1, NTILES - 1):
                g = k % NG
                xt = xpool.tile([128, T], f32, tag="xt", name=f"xt{k}")
                nc.sync.dma_start(out=xt[:, :], in_=x[k * 128:(k + 1) * 128, :])
                yt = ypool.tile([128, T], f32, tag="yt", name=f"yt{k}")
                nc.vector.tensor_tensor_scan(
                    yt[:, 0:H], bcast(g, H), xt[:, 0:H], 0.0, mult, add)
                nc.scalar.dma_start(out=y[k * 128:(k + 1) * 128, 0:H],
                                    in_=yt[:, 0:H])
                nc.vector.tensor_tensor_scan(
                    yt[:, H:T], bcast(g, H), xt[:, H:T],
                    yt[:, H - 1:H], mult, add)
                nc.scalar.dma_start(out=y[k * 128:(k + 1) * 128, H:T],
                                    in_=yt[:, H:T])

            # ---- block 7: chunked stores so the tail is short ----
            k = NTILES - 1
            xt7 = xpool.tile([128, T], f32, tag="xt", name="xt7")
            nc.sync.dma_start(out=xt7[:, :], in_=x[k * 128:(k + 1) * 128, :])
            yt7 = ypool.tile([128, T], f32, tag="yt", name="yt7")
            g = k % NG
            # shrinking chunks so the very last store is only 256KB
            bounds7 = [0, 1536, 2560, 3584, T]
            for c in range(len(bounds7) - 1):
                lo, hi = bounds7[c], bounds7[c + 1]
                init = 0.0 if c == 0 else yt7[:, lo - 1:lo]
                nc.vector.tensor_tensor_scan(
                    yt7[:, lo:hi], bcast(g, hi - lo), xt7[:, lo:hi],
                    init, mult, add)
                nc.scalar.dma_start(out=y[k * 128:(k + 1) * 128, lo:hi],
                                    in_=yt7[:, lo:hi])

    nc.compile()
    return nc


def _get_nc():
    global _cached
    if _cached is None:
        _cached = _build()
    return _cached


def _make_in_maps(data, tau_syn):
    tau = np.asarray(tau_syn, dtype=np.float64)
    beta = np.exp(-DT / tau).astype(np.float32)  # (1, N)
    beta_g = np.ascontiguousarray(beta.reshape(NG, 128).T)  # (128, NG)
    # (B, T, N) -> (B, N, T), batch-sharded across cores
    xt = np.ascontiguousarray(np.asarray(data, dtype=np.float32).transpose(0, 2, 1))
    xt = xt.reshape(NCORES, ROWS, T)
    return [{"x": xt[c], "beta": beta_g} for c in range(NCORES)]


def kernel(data, tau_syn):
    from concourse.bass_utils import run_bass_kernel_spmd

    nc = _get_nc()
    in_maps = _make_in_maps(data, tau_syn)
    res = run_bass_kernel_spmd(nc, in_maps, list(range(NCORES)))
    out = np.stack([res.results[c]["y"] for c in range(NCORES)])  # (8, ROWS, T)
    out = out.reshape(B, N, T).transpose(0, 2, 1)  # (B, T, N)
    return np.ascontiguousarray(out)
